# revision 28
# baseline (speedup 1.0000x reference)
"""Distributed Bass kernel for nn_Attention (B=2,T=2048,E=1024,H=16) on 8 trn2 cores.

Sharding: core c owns heads {2c, 2c+1} for BOTH batches (tensor parallel on qkv
columns). After attention, a single 8-core AllToAll per head redistributes the
head-sharded output to (batch, t-slice)-sharded layout for the FFN: core c runs
the FFN for batch c//4, t-rows [512*(c%4), 512*(c%4+1)).

Attention math (same trick as the original): the reference multiplies raw scores
by a lower-triangular 0/1 mask BEFORE softmax (masked logits -> exp(0)=1). Key
blocks strictly above the diagonal have P==1 exactly; their contribution (suffix
sums of V plus a count for the denominator) is injected with one small matmul
per q-block using host-provided constants. Only lower-triangular + diagonal
score blocks are computed/exp'd.

Layouts: QK are produced transposed ([qk-dim, t]); the P*V product is computed
"swapped" (out = P.T @ Vaug giving [q, 65] tiles: 64 V-features + the softmax
denominator Z), so normalization is a per-partition scalar multiply. Normalized
tiles are PE-transposed back to [feat, t] for the AllToAll / FFN. FFN1 runs in
two 4-matmul halves (head-0 rows after the first AllToAll, head-1 rows after the
second) against host-row-gathered W1 slices, keeping full 128-deep contraction.
"""

import numpy as np
import ml_dtypes

BF16 = ml_dtypes.bfloat16

B, T, E, H = 2, 2048, 1024, 16
DH = 64
NCORES = 8
TQ = 512         # FFN t-rows per core
NT = T // 128    # 16 t-blocks
NE = E // 128    # 8 E-tiles
CH = 512         # q-chunk size
NCH = T // CH    # 4 chunks

_NC_CACHE = {}


def _build_nc():
    import concourse.bass as bass
    import concourse.mybir as mybir
    import concourse.tile as tile
    from concourse import bacc

    fp32 = mybir.dt.float32
    bf16 = mybir.dt.bfloat16
    Exp = mybir.ActivationFunctionType.Exp
    Relu = mybir.ActivationFunctionType.Relu
    Copy = mybir.ActivationFunctionType.Copy
    Ident = mybir.ActivationFunctionType.Identity
    add = mybir.AluOpType.add
    mult = mybir.AluOpType.mult

    nc = bacc.Bacc(None, target_bir_lowering=False)

    # ---- parameters (per-core shards staged by host) ----
    xt0_p = nc.declare_dram_parameter("xt0", [E, T], bf16, isOutput=False)  # x[0].T
    xt1_p = nc.declare_dram_parameter("xt1", [E, T], bf16, isOutput=False)  # x[1].T
    wqk_p = nc.declare_dram_parameter("wqk", [E, 256], bf16, isOutput=False)  # [Q h0|h1, K h0|h1]
    wv_p = nc.declare_dram_parameter("wv", [E, 128], bf16, isOutput=False)   # [v h0|h1]
    bqk_p = nc.declare_dram_parameter("bqk", [128, 2], fp32, isOutput=False)
    c1_p = nc.declare_dram_parameter("c1t", [128, 8], fp32, isOutput=False)  # W1.T@bv + b1
    b2_p = nc.declare_dram_parameter("b2t", [128, 8], fp32, isOutput=False)
    w1h0_p = nc.declare_dram_parameter("w1h0", [512, E], bf16, isOutput=False)  # W1 rows, h0-gathered
    w1h1_p = nc.declare_dram_parameter("w1h1", [512, E], bf16, isOutput=False)
    w2_p = nc.declare_dram_parameter("w2", [E, E], bf16, isOutput=False)
    mask_p = nc.declare_dram_parameter("maskc", [128, 128], mybir.dt.uint8, isOutput=False)
    ones_p = nc.declare_dram_parameter("onesc", [128, 128], bf16, isOutput=False)
    idc_p = nc.declare_dram_parameter("idc", [128, 128], fp32, isOutput=False)
    ind_p = nc.declare_dram_parameter("indc", [16, T], bf16, isOutput=False)
    sufc_p = nc.declare_dram_parameter("sufc", [128, 256], bf16, isOutput=False)
    out_p = nc.declare_dram_parameter("out", [E, TQ], fp32, isOutput=True)    # final out.T slice

    with tile.TileContext(nc) as tc:
        with (
            tc.tile_pool(name="const", bufs=1) as cpool,
            tc.tile_pool(name="wts", bufs=1) as wpool,
            tc.tile_pool(name="xt", bufs=1) as xpool,
            tc.tile_pool(name="qk", bufs=1) as qkpool,
            tc.tile_pool(name="vaug", bufs=1) as vpool,
            tc.tile_pool(name="vsuf", bufs=1) as vspool,
            tc.tile_pool(name="ptile", bufs=3) as ppool,
            tc.tile_pool(name="an", bufs=3) as napool,
            tc.tile_pool(name="aT", bufs=1) as atpool,
            tc.tile_pool(name="rz", bufs=2) as rpool,
            tc.tile_pool(name="ffn", bufs=1) as fpool,
            tc.tile_pool(name="ot", bufs=2) as opool,
            tc.tile_pool(name="dram", bufs=1, space="DRAM") as dpool,
            tc.tile_pool(name="ps", bufs=2, space="PSUM") as pspool,
        ):
            # ---- input DMAs split across the SP and ACT hwdge queues so the
            # QKV(b0) inputs land as fast as possible ----
            bqk = cpool.tile([128, 2], fp32, tag="bqk", name="bqk")
            nc.sync.dma_start(out=bqk[:, :], in_=bqk_p[:, :])
            wqk = []
            xts = [[], []]
            for et in range(NE):
                t3 = xpool.tile([128, T], bf16, tag=f"xt0{et}", name=f"xt0{et}")
                eng = nc.sync if et % 2 == 0 else nc.scalar
                eng.dma_start(out=t3[:, :], in_=xt0_p[et * 128:(et + 1) * 128, :])
                xts[0].append(t3)
                t1 = wpool.tile([128, 256], bf16, tag=f"wqk{et}", name=f"wqk{et}")
                nc.sync.dma_start(out=t1[:, :], in_=wqk_p[et * 128:(et + 1) * 128, :])
                wqk.append(t1)
            wv = []
            for et in range(NE):
                t2 = wpool.tile([128, 128], bf16, tag=f"wv{et}", name=f"wv{et}")
                nc.scalar.dma_start(out=t2[:, :], in_=wv_p[et * 128:(et + 1) * 128, :])
                wv.append(t2)
            sufc = cpool.tile([128, 256], bf16, tag="sufc", name="sufc")
            nc.sync.dma_start(out=sufc[:, :], in_=sufc_p[:, :])
            indc = cpool.tile([16, T], bf16, tag="indc", name="indc")
            nc.sync.dma_start(out=indc[:, :], in_=ind_p[:, :])
            maskc = cpool.tile([128, 128], mybir.dt.uint8, tag="maskc", name="maskc")
            nc.sync.dma_start(out=maskc[:, :], in_=mask_p[:, :])
            onesc = cpool.tile([128, 128], bf16, tag="onesc", name="onesc")
            nc.sync.dma_start(out=onesc[:, :], in_=ones_p[:, :])
            idc = cpool.tile([128, 128], fp32, tag="idc", name="idc")
            nc.sync.dma_start(out=idc[:, :], in_=idc_p[:, :])
            for et in range(NE):
                t3 = xpool.tile([128, T], bf16, tag=f"xt1{et}", name=f"xt1{et}")
                eng = nc.sync if et % 2 == 0 else nc.scalar
                eng.dma_start(out=t3[:, :], in_=xt1_p[et * 128:(et + 1) * 128, :])
                xts[1].append(t3)
            c1t = cpool.tile([128, 8], fp32, tag="c1t", name="c1t")
            nc.sync.dma_start(out=c1t[:, :], in_=c1_p[:, :])
            b2t = cpool.tile([128, 8], fp32, tag="b2t", name="b2t")
            nc.sync.dma_start(out=b2t[:, :], in_=b2_p[:, :])
            w1h0 = []
            w1h1 = []
            w2sb = []
            for j in range(4):
                t4 = wpool.tile([128, E], bf16, tag=f"w1h0{j}", name=f"w1h0{j}")
                nc.sync.dma_start(out=t4[:, :], in_=w1h0_p[j * 128:(j + 1) * 128, :])
                w1h0.append(t4)
            for j in range(4):
                t5 = wpool.tile([128, E], bf16, tag=f"w1h1{j}", name=f"w1h1{j}")
                nc.sync.dma_start(out=t5[:, :], in_=w1h1_p[j * 128:(j + 1) * 128, :])
                w1h1.append(t5)
            for et in range(NE):
                t6 = wpool.tile([128, E], bf16, tag=f"w2{et}", name=f"w2{et}")
                nc.sync.dma_start(out=t6[:, :], in_=w2_p[et * 128:(et + 1) * 128, :])
                w2sb.append(t6)

            # qt/kt per batch: [128, T] bf16, partitions 0-63 = head0 dims, 64-127 = head1
            qkt = [[qkpool.tile([128, T], bf16, tag=f"qkt{b}{g}", name=f"qkt{b}{g}")
                    for g in range(2)] for b in range(2)]
            vaug = [[], []]   # per batch: 16 tiles [128, 130] (2 heads x 65)
            vsufb = [None, None]  # per batch: [16, 130] bf16

            def emit_qk1(b, g, half):
                # One Q-or-K et-outer pass covering chunk-pair `half`
                ps = pspool.tile([128, 2 * CH], fp32, tag="st", bufs=3, name=f"qkps{b}{g}{half}")
                for et in range(NE):
                    for i in range(2):
                        tch = half * 2 + i
                        nc.tensor.matmul(
                            ps[:, i * CH:(i + 1) * CH],
                            lhsT=wqk[et][:, g * 128:(g + 1) * 128],
                            rhs=xts[b][et][:, tch * CH:(tch + 1) * CH],
                            start=(et == 0), stop=(et == NE - 1),
                        )
                for i in range(2):
                    tch = half * 2 + i
                    nc.vector.tensor_scalar(
                        out=qkt[b][g][:, tch * CH:(tch + 1) * CH],
                        in0=ps[:, i * CH:(i + 1) * CH],
                        scalar1=bqk[:, g:g + 1], scalar2=None, op0=add,
                    )

            def emit_v_alloc(b):
                for tt in range(NT):
                    va = vpool.tile([128, 130], bf16, tag=f"va{b}{tt}", name=f"va{b}{tt}")
                    vaug[b].append(va)
                    va3 = va[:, :].rearrange("p (h c) -> p h c", c=65)
                    nc.gpsimd.memset(va3[:, :, 64:65], 1.0)

            def emit_v(b, pair):
                # V for 2 t-blocks, one accumulation group per psum bank
                vp = pspool.tile([128, 2 * CH], fp32, tag="st", bufs=3, name=f"vps{b}{pair}")
                for et in range(NE):
                    for i in range(2):
                        tt = pair * 2 + i
                        nc.tensor.matmul(
                            vp[:, i * CH:i * CH + 128],
                            lhsT=xts[b][et][:, tt * 128:(tt + 1) * 128],
                            rhs=wv[et][:, :],
                            start=(et == 0), stop=(et == NE - 1),
                        )
                for i in range(2):
                    tt = pair * 2 + i
                    va3 = vaug[b][tt][:, :].rearrange("p (h c) -> p h c", c=65)
                    nc.vector.tensor_copy(
                        va3[:, :, 0:64],
                        vp[:, i * CH:i * CH + 128].rearrange("p (h d) -> p h d", d=64),
                    )

            def emit_vsuf(b):
                # V suffix block sums (both heads + ones col)
                vsp = pspool.tile([16, 130], fp32, tag="st", bufs=3, name=f"vsp{b}")
                for tt in range(NT):
                    nc.tensor.matmul(
                        vsp[0:16, :],
                        lhsT=sufc[:, tt * 16:(tt + 1) * 16],
                        rhs=vaug[b][tt][:, 0:130],
                        start=(tt == 0), stop=(tt == NT - 1),
                    )
                vsb = vspool.tile([16, 130], bf16, tag=f"vsuf{b}", name=f"vsuf{b}")
                nc.vector.tensor_copy(vsb[:, :], vsp[:, :])
                vsufb[b] = vsb

            # Filler queue: QKV passes drained between attention score pairs
            # so the tensor engine keeps pace with the (slower) exp stream.
            fq = []
            fdone = set()

            def drain_one():
                if fq:
                    nm, fn = fq.pop(0)
                    fn()
                    fdone.add(nm)

            def drain_until(name):
                while name not in fdone and fq:
                    drain_one()

            # aT[h][b]: [64, T] bf16 transposed attention output (feat, t)
            aT = [[atpool.tile([64, T], bf16, tag=f"aT{h}{b}", name=f"aT{h}{b}")
                   for b in range(2)] for h in range(2)]
            a2a_in = [dpool.tile([512, 512], bf16, tag=f"a2ain{h}", name=f"a2ain{h}")
                      for h in range(2)]
            a2a_out = [dpool.tile([512, 512], bf16, tag=f"a2aout{h}", name=f"a2aout{h}")
                       for h in range(2)]

            accs = {}

            def emit_attn_chunk(h, b, qc):
                """Scores + exp + diag mask + P.T@Vaug for one q-chunk.
                The accumulator stays open; emit_fin injects the suffix and
                closes it."""
                qt = qkt[b][0]
                kt = qkt[b][1]
                p0 = 64 * h
                tpos = (p0, 0)
                q0 = qc * CH
                acc = pspool.tile([128, 260], fp32, tag="acc", name=f"acc{h}{b}{qc}")
                accs[(h, b, qc)] = acc
                nkj = 4 * qc + 4

                def do_pv(pair):
                    k0, k1, n0, n1, qoff0, qoff1, st = pair
                    p2 = ppool.tile([128, 2 * CH], bf16, tag="p2", name=f"p2{h}{b}{qc}{k0}")
                    if n0 == CH:
                        nc.scalar.activation(p2[:, 0:CH + n1], st[:, 0:CH + n1], Exp, scale=0.125)
                    else:
                        nc.scalar.activation(p2[:, 0:n0], st[:, 0:n0], Exp, scale=0.125)
                        nc.scalar.activation(p2[:, CH:CH + n1], st[:, CH:CH + n1], Exp, scale=0.125)
                    for kj, off in ((k0, 0), (k1, CH)):
                        if kj * 128 >= q0:
                            # diagonal block: masked (k > q) -> exp(0) = 1
                            nc.vector.copy_predicated(
                                out=p2[:, off:off + 128],
                                mask=maskc[:, :],
                                data=onesc[:, :],
                            )
                    for kj, off, qoff in ((k0, 0, qoff0), (k1, CH, qoff1)):
                        qb_lo = (qoff - q0) // 128
                        for qb in range(qb_lo, 4):
                            c = 128 * qb - (qoff - q0)
                            nc.tensor.matmul(
                                acc[:, qb * 65:qb * 65 + 65],
                                lhsT=p2[:, off + c:off + c + 128],
                                rhs=vaug[b][kj][:, 65 * h:65 * h + 65],
                                start=(kj == 0 and qb == 0), stop=False,
                            )

                pending = []
                for k0 in range(0, nkj, 2):
                    k1 = k0 + 1
                    qoff0 = max(k0 * 128, q0)
                    qoff1 = max(k1 * 128, q0)
                    n0 = q0 + CH - qoff0
                    n1 = q0 + CH - qoff1
                    st = pspool.tile([128, 2 * CH], fp32, tag="st", bufs=3, name=f"st{h}{b}{qc}{k0}")
                    nc.tensor.matmul(
                        st[:, 0:n0],
                        lhsT=kt[p0:p0 + 64, k0 * 128:(k0 + 1) * 128],
                        rhs=qt[p0:p0 + 64, qoff0:q0 + CH],
                        start=True, stop=True, tile_position=tpos,
                    )
                    nc.tensor.matmul(
                        st[:, CH:CH + n1],
                        lhsT=kt[p0:p0 + 64, k1 * 128:(k1 + 1) * 128],
                        rhs=qt[p0:p0 + 64, qoff1:q0 + CH],
                        start=True, stop=True, tile_position=tpos,
                    )
                    pending.append((k0, k1, n0, n1, qoff0, qoff1, st))
                    if len(pending) > 2:
                        do_pv(pending.pop(0))
                    if k0 % 4 == 2:
                        drain_one()
                for pair in pending:
                    do_pv(pair)

            def emit_fin(h, b, qc):
                """Close a chunk: suffix injection, normalize, transpose."""
                q0 = qc * CH
                acc = accs.pop((h, b, qc))
                for qb in range(4):
                    nc.tensor.matmul(
                        acc[:, qb * 65:qb * 65 + 65],
                        lhsT=indc[:, q0 + 128 * qb:q0 + 128 * qb + 128],
                        rhs=vsufb[b][:, 65 * h:65 * h + 65],
                        start=False, stop=(qb == 3),
                    )
                rz = rpool.tile([128, 4], fp32, tag="rz", name=f"rz{h}{b}{qc}")
                acc3 = acc[:, :].rearrange("p (q c) -> p q c", c=65)
                rz3 = rz[:, :].rearrange("p (q c) -> p q c", c=1)
                nc.vector.reciprocal(rz3[:, :, :], acc3[:, :, 64:65])
                tp = pspool.tile([64, 2 * CH], fp32, tag="st", bufs=3, name=f"tp{h}{b}{qc}")
                tp3 = tp[:, :].rearrange("p (s c) -> p s c", s=2)
                for half in range(2):
                    for i in range(2):
                        qb = half * 2 + i
                        an = napool.tile([128, 64], fp32, tag="an", name=f"an{h}{b}{qc}{qb}")
                        nc.vector.tensor_scalar(
                            out=an[:, :], in0=acc[:, qb * 65:qb * 65 + 64],
                            scalar1=rz[:, qb:qb + 1], scalar2=None, op0=mult,
                        )
                        nc.tensor.transpose(tp3[:, i, 0:128], an[:, :], idc[:, :])
                    nc.vector.tensor_copy(
                        aT[h][b][:, q0 + 256 * half:q0 + 256 * half + 256]
                        .rearrange("p (s c) -> p s c", s=2),
                        tp3[:, :, 0:128],
                    )

            def emit_a2a_in(h, b):
                # destination rank d = 4*b + s gets rows [64d, 64d+64) = slice s
                for s in range(4):
                    nc.sync.dma_start(
                        out=a2a_in[h][64 * (4 * b + s):64 * (4 * b + s) + 64, :],
                        in_=aT[h][b][:, 512 * s:512 * s + 512],
                    )

            def emit_a2a(h):
                nc.gpsimd.collective_compute(
                    "AllToAll",
                    mybir.AluOpType.bypass,
                    ins=[a2a_in[h][:, :].opt()],
                    outs=[a2a_out[h][:, :].opt()],
                    replica_groups=[[0, 1, 2, 3, 4, 5, 6, 7]],
                )

            emit_v_alloc(0)
            emit_v_alloc(1)
            emit_qk1(0, 0, 0)
            emit_qk1(0, 1, 0)
            emit_v(0, 0)
            emit_v(0, 1)
            fq.extend([
                ("v02", lambda: emit_v(0, 2)),
                ("v03", lambda: emit_v(0, 3)),
                ("v04", lambda: emit_v(0, 4)),
                ("v05", lambda: emit_v(0, 5)),
                ("v06", lambda: emit_v(0, 6)),
                ("v07", lambda: emit_v(0, 7)),
                ("vsuf0", lambda: emit_vsuf(0)),
                ("qk001", lambda: emit_qk1(0, 0, 1)),
                ("qk011", lambda: emit_qk1(0, 1, 1)),
                ("qk100", lambda: emit_qk1(1, 0, 0)),
                ("qk110", lambda: emit_qk1(1, 1, 0)),
                ("v10", lambda: emit_v(1, 0)),
                ("v11", lambda: emit_v(1, 1)),
                ("v12", lambda: emit_v(1, 2)),
                ("v13", lambda: emit_v(1, 3)),
                ("qk101", lambda: emit_qk1(1, 0, 1)),
                ("qk111", lambda: emit_qk1(1, 1, 1)),
                ("v14", lambda: emit_v(1, 4)),
                ("v15", lambda: emit_v(1, 5)),
                ("v16", lambda: emit_v(1, 6)),
                ("v17", lambda: emit_v(1, 7)),
                ("vsuf1", lambda: emit_vsuf(1)),
            ])
            emit_attn_chunk(0, 0, 0)
            drain_until("v03")
            emit_attn_chunk(0, 0, 1)
            drain_until("vsuf0")
            emit_fin(0, 0, 0)
            drain_until("qk011")
            emit_attn_chunk(0, 0, 2)
            emit_fin(0, 0, 1)
            emit_attn_chunk(0, 0, 3)
            emit_fin(0, 0, 2)
            emit_fin(0, 0, 3)
            emit_a2a_in(0, 0)
            drain_until("v11")
            emit_attn_chunk(0, 1, 0)
            drain_until("v13")
            emit_attn_chunk(0, 1, 1)
            drain_until("vsuf1")
            emit_fin(0, 1, 0)
            emit_attn_chunk(0, 1, 2)
            emit_fin(0, 1, 1)
            emit_attn_chunk(0, 1, 3)
            emit_fin(0, 1, 2)
            emit_fin(0, 1, 3)
            emit_a2a_in(0, 1)
            emit_a2a(0)
            emit_attn_chunk(1, 0, 0)
            emit_attn_chunk(1, 0, 1)
            emit_fin(1, 0, 0)
            emit_attn_chunk(1, 0, 2)
            emit_fin(1, 0, 1)
            emit_attn_chunk(1, 0, 3)
            emit_fin(1, 0, 2)
            emit_fin(1, 0, 3)
            emit_a2a_in(1, 0)
            emit_attn_chunk(1, 1, 0)
            emit_attn_chunk(1, 1, 1)
            emit_fin(1, 1, 0)
            emit_attn_chunk(1, 1, 2)
            emit_fin(1, 1, 1)
            emit_attn_chunk(1, 1, 3)
            emit_fin(1, 1, 2)
            emit_fin(1, 1, 3)
            emit_a2a_in(1, 1)
            emit_a2a(1)

            # ---- FFN ----
            # agt0[j] = a2a_out0 rows [128j, 128j+128) (h0 features of ranks 2j, 2j+1)
            agt0 = []
            for j in range(4):
                t7 = fpool.tile([128, TQ], bf16, tag=f"agt0{j}", name=f"agt0{j}")
                nc.sync.dma_start(out=t7[:, :], in_=a2a_out[0][j * 128:(j + 1) * 128, :])
                agt0.append(t7)
            # FFN1 head0 partial (overlaps the second AllToAll), evac to SBUF fp32
            h0p = []
            for e1 in range(NE):
                ps = pspool.tile([128, CH], fp32, tag="acc", name=f"f1a{e1}")
                for j in range(4):
                    nc.tensor.matmul(
                        ps[:, :],
                        lhsT=w1h0[j][:, e1 * 128:(e1 + 1) * 128],
                        rhs=agt0[j][:, :],
                        start=(j == 0), stop=(j == 3),
                    )
                hp = fpool.tile([128, CH], fp32, tag=f"h0p{e1}", name=f"h0p{e1}")
                nc.vector.tensor_copy(hp[:, :], ps[:, :])
                h0p.append(hp)
            agt1 = []
            for j in range(4):
                t8 = fpool.tile([128, TQ], bf16, tag=f"agt1{j}", name=f"agt1{j}")
                eng = nc.scalar if j % 2 == 0 else nc.sync
                eng.dma_start(out=t8[:, :], in_=a2a_out[1][j * 128:(j + 1) * 128, :])
                agt1.append(t8)
            # FFN1 head1 partial + combine + relu
            h1t = []
            for e1 in range(NE):
                ps = pspool.tile([128, CH], fp32, tag="acc", name=f"f1b{e1}")
                for j in range(4):
                    nc.tensor.matmul(
                        ps[:, :],
                        lhsT=w1h1[j][:, e1 * 128:(e1 + 1) * 128],
                        rhs=agt1[j][:, :],
                        start=(j == 0), stop=(j == 3),
                    )
                hs = napool.tile([128, CH], fp32, tag="hsum", name=f"hsum{e1}")
                nc.vector.tensor_tensor(out=hs[:, :], in0=ps[:, :], in1=h0p[e1][:, :], op=add)
                ht = fpool.tile([128, CH], bf16, tag=f"h1t{e1}", name=f"h1t{e1}")
                nc.scalar.activation(ht[:, :], hs[:, :], Relu, bias=c1t[:, e1:e1 + 1])
                h1t.append(ht)
            # FFN2
            for e2 in range(NE):
                ps = pspool.tile([128, CH], fp32, tag="st", bufs=3, name=f"f2{e2}")
                for et in range(NE):
                    nc.tensor.matmul(
                        ps[:, :],
                        lhsT=w2sb[et][:, e2 * 128:(e2 + 1) * 128],
                        rhs=h1t[et][:, :],
                        start=(et == 0), stop=(et == NE - 1),
                    )
                ot = opool.tile([128, CH], fp32, tag="ot", name=f"ot{e2}")
                nc.vector.tensor_scalar(
                    out=ot[:, :], in0=ps[:, :],
                    scalar1=b2t[:, e2:e2 + 1], scalar2=None, op0=add,
                )
                nc.sync.dma_start(out=out_p[e2 * 128:(e2 + 1) * 128, :], in_=ot[:, :])

    nc.compile()
    return nc


def _host_prep(x, Wqkv, bqkv, W1, b1, W2, b2):
    """Build the 8 per-core input maps (numpy, host-side layout transforms)."""
    x = np.asarray(x, dtype=np.float32)
    Wqkv = np.asarray(Wqkv, dtype=np.float32)
    bqkv = np.asarray(bqkv, dtype=np.float32)
    W1 = np.asarray(W1, dtype=np.float32)
    b1 = np.asarray(b1, dtype=np.float32)
    W2 = np.asarray(W2, dtype=np.float32)
    b2 = np.asarray(b2, dtype=np.float32)

    # attention-output bias per E index (head-major): bv_full[e] = bqkv[h*192+128+d]
    ei = np.arange(E)
    bv_full = bqkv[(ei // DH) * 3 * DH + 2 * DH + (ei % DH)]
    c1 = W1.T @ bv_full + b1
    c1t = np.ascontiguousarray(c1.reshape(8, 128).T, dtype=np.float32)   # (128, 8)
    b2t = np.ascontiguousarray(b2.reshape(8, 128).T, dtype=np.float32)

    kk, qq = np.meshgrid(np.arange(128), np.arange(128), indexing="ij")
    maskc = (kk > qq).astype(np.uint8)     # 1 where masked (k > q)
    onesc = np.ones((128, 128), dtype=BF16)
    idc = np.eye(128, dtype=np.float32)
    jj, tq = np.meshgrid(np.arange(16), np.arange(T), indexing="ij")
    indc = (jj == tq // 128 + 1).astype(BF16)                             # (16, T)
    ttj = np.zeros((128, 256), dtype=np.float32)
    for tt in range(16):
        for j in range(16):
            if j <= tt:
                ttj[:, tt * 16 + j] = 1.0
    sufc = ttj.astype(BF16)                                               # (128, 256)

    # W1 rows gathered to match AllToAll output row order per head half
    r = np.arange(512)
    perm0 = 128 * (r // 64) + (r % 64)
    perm1 = perm0 + 64
    w1h0 = np.ascontiguousarray(W1[perm0, :], dtype=BF16)
    w1h1 = np.ascontiguousarray(W1[perm1, :], dtype=BF16)
    w2b = W2.astype(BF16)
    xt = [np.ascontiguousarray(x[b].T, dtype=BF16) for b in range(B)]

    in_maps = []
    for c in range(NCORES):
        h0, h1 = 2 * c, 2 * c + 1
        qcols = lambda h: np.arange(h * 3 * DH, h * 3 * DH + DH)
        kcols = lambda h: np.arange(h * 3 * DH + DH, h * 3 * DH + 2 * DH)
        vcols = lambda h: np.arange(h * 3 * DH + 2 * DH, h * 3 * DH + 3 * DH)
        qksel = np.concatenate([qcols(h0), qcols(h1), kcols(h0), kcols(h1)])
        wqk = np.ascontiguousarray(Wqkv[:, qksel], dtype=BF16)            # (E, 256)
        bqk = np.ascontiguousarray(bqkv[qksel].reshape(2, 128).T, dtype=np.float32)  # (128, 2)
        vsel = np.concatenate([vcols(h0), vcols(h1)])
        wv = np.ascontiguousarray(Wqkv[:, vsel], dtype=BF16)              # (E, 128)
        in_maps.append({
            "xt0": xt[0], "xt1": xt[1], "wqk": wqk, "wv": wv, "bqk": bqk,
            "c1t": c1t, "b2t": b2t, "w1h0": w1h0, "w1h1": w1h1, "w2": w2b,
            "maskc": maskc, "onesc": onesc, "idc": idc, "indc": indc, "sufc": sufc,
        })
    return in_maps


TRACE = False
LAST_EXEC_NS = None
LAST_RESULTS = None


def kernel(x, Wqkv, bqkv, W1, b1, W2, b2, n_heads=16):
    global LAST_EXEC_NS, LAST_RESULTS
    from concourse import bass_utils

    if "nc" not in _NC_CACHE:
        _NC_CACHE["nc"] = _build_nc()
    nc = _NC_CACHE["nc"]

    in_maps = _host_prep(x, Wqkv, bqkv, W1, b1, W2, b2)
    try:
        res = bass_utils.run_bass_kernel_spmd(
            nc, in_maps, core_ids=list(range(NCORES)), trace=TRACE,
        )
    except ModuleNotFoundError:
        res = bass_utils.run_bass_kernel_spmd(
            nc, in_maps, core_ids=list(range(NCORES)), trace=False,
        )
    LAST_EXEC_NS = res.exec_time_ns
    LAST_RESULTS = res
    y = np.empty((B, T, E), dtype=np.float32)
    for c in range(NCORES):
        b, g = c // 4, c % 4
        outT = np.asarray(res.results[c]["out"], dtype=np.float32)   # (E, TQ)
        y[b, g * TQ:(g + 1) * TQ, :] = outT.T
    return y
